# revision 1
# baseline (speedup 1.0000x reference)
"""Trainium2 Bass kernel for nn_Attention_structure_76072460747267.

Sharding: data-parallel over batch — 8 batch items onto 8 NeuronCores, no
collectives. Per core, the full attention layer for one [1024, 512] item.

Device layout ("ji" / key-major attention):
  - Host passes x^T [512, 1024] so all matmuls contract over partitions.
  - Q^T, K^T computed feature-major per head [64, 1024]; V token-major
    [j, 64h] with a ones-column appended per head, so attn@V also yields the
    softmax denominator row for free (row 64 of each [65, 512] PSUM tile).
  - The dist->conv1->relu->conv2 bias is a pointwise scalar function of
    dist[i, j] only; it is precomputed on host (bf16, transposed to [h, j, i])
    and added into the dots PSUM via an identity matmul while DMA overlaps
    with PE compute.
  - Softmax without max-subtraction (dots+bias are O(1) by construction).
  - Normalization is deferred through attn@V: out^T rows are divided by the
    sumexp row (vector.reciprocal + gpsimd partition_broadcast) before the
    final W_out projection; b_out is added via scalar_tensor_tensor.
"""

import sys

sys.path.insert(0, "/opt/trn_rl_repo")

import numpy as np
import ml_dtypes

from contextlib import ExitStack

from concourse import bass, mybir, tile
from concourse.bass_utils import run_bass_kernel_spmd

F32 = mybir.dt.float32
F32R = mybir.dt.float32r
BF16 = mybir.dt.bfloat16

DIM = 512
N = 1024
HEADS = 8
DH = 64
SCALE = DH**-0.5

_CACHED_NC = None
_last_in_maps = None


def _split_waits(nc):
    """Walrus codegen in this environment accepts at most ONE sync-wait per
    instruction. Tile sometimes emits 2+. Split the extras onto same-engine
    NoOps placed immediately before the instruction (engine program order
    guarantees they complete first)."""
    n_split = 0
    for fn in nc.m.functions:
        for bb in fn.blocks:
            out = []
            for inst in bb.instructions:
                si = getattr(inst, "sync_info", None)
                waits = list(si.on_wait) if si is not None and si.on_wait else []
                if len(waits) > 1:
                    for k, w in enumerate(waits[:-1]):
                        nop = mybir.InstNoOp(
                            name=f"{inst.name}_sw{k}",
                            engine=inst.engine,
                            sync_info=mybir.SyncInfo(on_wait=[w], on_update=[]),
                            bass_nofuse=True,
                        )
                        out.append(nop)
                        n_split += 1
                    inst.sync_info = mybir.SyncInfo(
                        on_wait=[waits[-1]], on_update=list(si.on_update or [])
                    )
                out.append(inst)
            try:
                bb.instructions = out
            except Exception:
                bb.instructions.clear()
                bb.instructions.extend(out)
    return n_split


def _build_nc():
    nc = bass.Bass("TRN2", target_bir_lowering=False, debug=False)

    xT_d = nc.dram_tensor("xT", [DIM, N], BF16, kind="ExternalInput").ap()
    biasT_d = nc.dram_tensor("biasT", [HEADS, N, N], BF16, kind="ExternalInput").ap()
    wpack_d = nc.dram_tensor("wpack", [DIM, 3 * DIM], BF16, kind="ExternalInput").ap()
    wout_d = nc.dram_tensor("wout", [DIM, DIM], BF16, kind="ExternalInput").ap()
    bout_d = nc.dram_tensor("bout", [128, DIM], F32, kind="ExternalInput").ap()
    ident_d = nc.dram_tensor("ident", [128, 128], BF16, kind="ExternalInput").ap()
    out_d = nc.dram_tensor("out", [N, DIM], F32, kind="ExternalOutput").ap()

    with tile.TileContext(nc) as tc, ExitStack() as ctx:
        const = ctx.enter_context(tc.tile_pool(name="const", bufs=1))
        biasp = ctx.enter_context(tc.tile_pool(name="biasp", bufs=4))
        expp = ctx.enter_context(tc.tile_pool(name="expp", bufs=16))
        rbp = ctx.enter_context(tc.tile_pool(name="rbp", bufs=3))
        outp = ctx.enter_context(tc.tile_pool(name="outp", bufs=3))
        psA = ctx.enter_context(tc.tile_pool(name="psA", bufs=2, space="PSUM"))
        psD = ctx.enter_context(tc.tile_pool(name="psD", bufs=3, space="PSUM"))
        psO = ctx.enter_context(tc.tile_pool(name="psO", bufs=2, space="PSUM"))
        psB = ctx.enter_context(tc.tile_pool(name="psB", bufs=1, space="PSUM"))

        # ---- persistent SBUF tensors -------------------------------------
        # xT packed [128, (c=4, i=1024)]; wpack [128, (c=4, 1536)]
        xT_sb = const.tile([128, 4 * N], BF16, tag="xT")
        wp_sb = const.tile([128, 4 * 3 * DIM], BF16, tag="wp")
        wo_sb = [const.tile([64, DIM], BF16, tag=f"wo{h}", name=f"wo{h}") for h in range(8)]
        ident_sb = const.tile([128, 128], BF16, tag="ident")
        onecol_sb = const.tile([1, 64], F32, tag="onecol")
        qT_sb = [const.tile([64, N], BF16, tag=f"qT{h}", name=f"qT{h}") for h in range(8)]
        kT_sb = [const.tile([64, N], BF16, tag=f"kT{h}", name=f"kT{h}") for h in range(8)]
        vaug_sb = [const.tile([128, 520], BF16, tag=f"va{j}", name=f"va{j}") for j in range(8)]
        ou_sb = const.tile([65, 16 * DIM], F32, tag="ou")
        on_sb = [const.tile([64, N], BF16, tag=f"on{h}", name=f"on{h}") for h in range(8)]
        sumr_sb = const.tile([16, DIM], F32, tag="sumr")
        recip_sb = const.tile([16, DIM], F32, tag="recip")
        bb_sb = const.tile([128, DIM], F32, tag="bb")

        nc.gpsimd.dma_start(xT_sb[:].rearrange("p (c i) -> p c i", c=4), xT_d.rearrange("(c p) i -> p c i", p=128))
        nc.gpsimd.dma_start(wp_sb[:].rearrange("p (c i) -> p c i", c=4), wpack_d.rearrange("(c p) i -> p c i", p=128))
        for h in range(8):
            nc.gpsimd.dma_start(wo_sb[h][:], wout_d[64 * h : 64 * h + 64, :])
        nc.gpsimd.dma_start(bb_sb[:], bout_d[:])
        nc.gpsimd.dma_start(ident_sb[:], ident_d[:])
        nc.vector.memset(onecol_sb[:], 1.0)

        def xT(c, lo, ln):
            return xT_sb[:, N * c + lo : N * c + lo + ln]

        def wslice(c, which, h):
            base = 1536 * c + 512 * which + 64 * h
            return wp_sb[:, base : base + 64]

        # ---- Phase 1: Q^T, K^T per head; V token-major, ones-augmented ---
        for h in range(HEADS):
            for ih in range(2):
                for wi, dst in ((0, qT_sb), (1, kT_sb)):
                    ps = psA.tile([128, 512], F32, tag="psA", name="psA_t")
                    for c in range(4):
                        nc.tensor.matmul(
                            ps[0:64, :], wslice(c, wi, h), xT(c, 512 * ih, 512),
                            start=(c == 0), stop=(c == 3),
                        )
                    nc.vector.tensor_copy(
                        dst[h][:, 512 * ih : 512 * ih + 512], ps[0:64, :]
                    )
        for jc in range(8):
            ps = psA.tile([128, 512], F32, tag="psA", name="psA_t")
            for c in range(4):
                nc.tensor.matmul(
                    ps[:],
                    xT(c, 128 * jc, 128),
                    wp_sb[:, 1536 * c + 1024 : 1536 * c + 1536],
                    start=(c == 0), stop=(c == 3),
                )
            nc.vector.memset(vaug_sb[jc][:], 1.0)
            dst3 = vaug_sb[jc][:].rearrange("p (h e) -> p h e", e=65)[:, :, 0:64]
            src3 = ps[:].rearrange("p (h e) -> p h e", e=64)
            nc.vector.tensor_copy(dst3, src3)

        # ---- Phase 2: dots + bias, exp, attn@V with ones-column ----------
        for h in range(HEADS):
            for ih in range(2):
                r = 2 * h + ih
                bt = biasp.tile([128, 8 * 512], BF16, tag="bt", name="bt_t")
                bsrc = biasT_d[h].rearrange("(c p) i -> p c i", p=128)[
                    :, :, 512 * ih : 512 * ih + 512
                ]
                nc.gpsimd.dma_start(bt[:].rearrange("p (c i) -> p c i", c=8), bsrc)
                ets = []
                for jc in range(8):
                    pd = psD.tile([128, 512], F32, tag="psD", name="psD_t")
                    nc.tensor.matmul(
                        pd[:],
                        kT_sb[h][:, 128 * jc : 128 * jc + 128],
                        qT_sb[h][:, 512 * ih : 512 * ih + 512],
                        start=True, stop=False,
                    )
                    nc.tensor.matmul(
                        pd[:], ident_sb[:], bt[:, 512 * jc : 512 * jc + 512],
                        start=False, stop=True, skip_group_check=True,
                    )
                    et = expp.tile([128, 512], BF16, tag="et", name="et_t")
                    nc.scalar.activation(
                        et[:], pd[:], mybir.ActivationFunctionType.Exp
                    )
                    ets.append(et)
                pot = psO.tile([128, 512], F32, tag="psO", name="psO_t")
                for jc in range(8):
                    nc.tensor.matmul(
                        pot[0:65, :],
                        vaug_sb[jc][:, 65 * h : 65 * h + 65],
                        ets[jc][:],
                        start=(jc == 0), stop=(jc == 7),
                    )
                nc.vector.tensor_copy(
                    ou_sb[:, 512 * r : 512 * r + 512], pot[0:65, :]
                )

        # ---- Phase 3: normalize, project, add b_out ----------------------
        nc.gpsimd.dma_start(
            sumr_sb[:].unsqueeze(0).rearrange("o r i -> (o r) i"), ou_sb[64:65, :].rearrange("o (r i) -> o r i", r=16)
        )
        nc.vector.reciprocal(recip_sb[:], sumr_sb[:])
        for h in range(HEADS):
            for ih in range(2):
                r = 2 * h + ih
                rrow = rbp.tile([1, 512], F32, tag="rrow", name="rrow_t")
                nc.gpsimd.dma_start(rrow[:], recip_sb[r : r + 1, :])
                rb = psB.tile([64, 512], F32, tag="rb", name="rb_t")
                nc.tensor.matmul(
                    rb[:], onecol_sb[:], rrow[:], start=True, stop=True
                )
                nc.vector.tensor_mul(
                    on_sb[h][:, 512 * ih : 512 * ih + 512],
                    ou_sb[0:64, 512 * r : 512 * r + 512],
                    rb[:],
                )
        for ic in range(8):
            pf = psA.tile([128, 512], F32, tag="psA", name="psA_t")
            for h in range(8):
                nc.tensor.matmul(
                    pf[:],
                    on_sb[h][:, 128 * ic : 128 * ic + 128],
                    wo_sb[h][:],
                    start=(h == 0), stop=(h == 7),
                )
            ot = outp.tile([128, 512], F32, tag="ot", name="ot_t")
            nc.vector.scalar_tensor_tensor(
                ot[:], pf[:], 1.0, bb_sb[:],
                op0=mybir.AluOpType.mult, op1=mybir.AluOpType.add,
            )
            nc.gpsimd.dma_start(out_d[128 * ic : 128 * ic + 128, :], ot[:])

    n = _split_waits(nc)
    print(f"_split_waits: {n} extra waits moved to NoOps", file=sys.stderr)
    return nc


def _host_bias(dist, c1w, c1b, c2w, c2b):
    """bias[b, h, j, i] (transposed!) in bf16, from dist [b, n, n] fp32."""
    b, n, _ = dist.shape
    d1 = (dist * (1.0 / 3.8)).astype(np.float32)
    f1 = 1.0 / (1.0 + d1)
    d2 = d1 * d1
    f2 = 1.0 / (1.0 + d2)
    f3 = 1.0 / (1.0 + d2 * d1)
    del d1, d2
    feats = np.stack([f1, f2, f3], axis=1).reshape(b, 3, n * n)
    del f1, f2, f3
    h1 = np.matmul(c1w.astype(np.float32), feats) + c1b[None, :, None]
    del feats
    np.maximum(h1, 0.0, out=h1)
    bias = np.matmul(c2w.astype(np.float32), h1) + c2b[None, :, None]
    del h1
    bias = bias.reshape(b, HEADS, n, n).transpose(0, 1, 3, 2)  # [b, h, j, i]
    return np.ascontiguousarray(bias).astype(ml_dtypes.bfloat16)


def kernel(**inputs):
    global _CACHED_NC, _last_in_maps
    x = np.asarray(inputs["x"], np.float32)
    dist = np.asarray(inputs["dist"], np.float32)
    W_qkv = np.asarray(inputs["W_qkv"], np.float32)
    W_out = np.asarray(inputs["W_out"], np.float32)
    b_out = np.asarray(inputs["b_out"], np.float32)
    c1w = np.asarray(inputs["conv1_w"], np.float32)
    c1b = np.asarray(inputs["conv1_b"], np.float32)
    c2w = np.asarray(inputs["conv2_w"], np.float32)
    c2b = np.asarray(inputs["conv2_b"], np.float32)

    b = x.shape[0]
    wpack = W_qkv.copy()
    wpack[:, :DIM] *= np.float32(SCALE)
    biasT = _host_bias(dist, c1w, c1b, c2w, c2b)
    ident = np.eye(128, dtype=ml_dtypes.bfloat16)
    bout2 = np.ascontiguousarray(np.broadcast_to(b_out.reshape(1, DIM), (128, DIM)))

    if _CACHED_NC is None:
        _CACHED_NC = _build_nc()
    nc = _CACHED_NC

    in_maps = []
    for i in range(b):
        in_maps.append(
            {
                "xT": np.ascontiguousarray(x[i].T).astype(ml_dtypes.bfloat16),
                "biasT": biasT[i],
                "wpack": wpack.astype(ml_dtypes.bfloat16),
                "wout": W_out.astype(ml_dtypes.bfloat16),
                "bout": bout2,
                "ident": ident,
            }
        )
    _last_in_maps = in_maps
    res = run_bass_kernel_spmd(nc, in_maps, list(range(b)))
    out = np.stack([res.results[i]["out"] for i in range(b)], axis=0)
    return out.astype(np.float32)

